# revision 39
# baseline (speedup 1.0000x reference)
"""Trainium2 Bass kernel for nn_CapsNet_69114613730132.

Strategy (8 NeuronCores, SPMD, n-axis tensor parallel per the sharding
hint):
  The CapsNet routing loop is degenerate (self.bij is never updated, so
  cij stays 1/512) and collapses to: conv1 -> conv2 -> squash ->
  4096->160 matvec -> elementwise squash.

  * conv1 (3->128, 9x9, on 24x24) is tiny and replicated: one
    243+1-contraction matmul over a host-built im2col (conv_b folded in
    as an extra contraction row).
  * conv2 / PrimaryCaps is sharded across cores on its 256 output
    channels (= the 512-input-capsule axis of the routing einsum): each
    core holds 32 output channels' weights (1/8 of pri_w, ~660KB bf16
    instead of 5.3MB replicated) and computes its (32, 16) slice of the
    primary capsule grid via 81 PSUM-accumulated tap matmuls over
    strided views of h (pri_b folded in as a rank-1 PE accumulation).
  * squash is applied per-core on its slice (groups of 8 lie along the
    free dim), then each core computes the partial DigitCaps sum over
    its 512-entry slice of the 4096-long contraction for ALL 160
    outputs (4 matmuls of 128-contraction x 160 cols; the constant
    cij=1/512 is folded into the weights).
  * The all-reduce over the input-capsule axis that the sharding hint
    prescribes is realized in the gather/unshard step: the host sums
    the 8 (1,160) partials and applies the final elementwise squash
    s*|s|/(1+s*s) (the routing output for a size-1 capsule dim).
  * All PE compute in bf16 (weights host-cast), f32 PSUM/vector math.

kernel(**inputs) takes the FULL unsharded inputs and returns the full
(1,1,10,16,1) float32 output.
"""
import numpy as np
import ml_dtypes

import concourse.bass as bass
import concourse.bacc as bacc
import concourse.tile as tile
import concourse.mybir as mybir
from concourse.bass_utils import run_bass_kernel_spmd
from concourse.tile import ScopedClock, add_dep_helper

FAST_TAIL = True


class FastTailTileContext(tile.TileContext):
    """TileContext tail with a 1-hop handshake instead of the all-engine
    barriers (each an EVSEM polling butterfly measured at ~7us here).

    The sync.drain waits for every tracked semaphore target, so by the
    time it passes, every sem-touching instruction on every engine has
    retired (each engine's last real work is upstream of the output DMA
    the drain waits on). A single drain->GpSimd semaphore hop then orders
    the sem/DMA-state clears; the next execution's NEFF entry barrier
    orders everything else."""

    def _drain_and_barrier(self, tick_clock, wait_clock):
        if not FAST_TAIL:
            return super()._drain_and_barrier(tick_clock, wait_clock)
        nc = self.nc
        # GpSimd (the clearing engine, otherwise idle here) waits on every
        # tracked semaphore's final value itself, then clears.
        drain_inst = nc.gpsimd.drain()
        wait_clock.add_sem_waits(
            drain_inst.ins, ScopedClock({None: tick_clock.global_clock})
        )
        # DMA completion sems update asynchronously and the final DMA (the
        # output store) has no downstream consumer, so the clock misses it:
        # wait each DMA-lane sem out to its summed final value explicitly.
        dma_totals = {}
        for insts in self.ordered_instructions_by_block.values():
            for i in insts:
                si = i.sync_info
                if si is None or not si.on_update:
                    continue
                for u in si.on_update:
                    if (u.sync_type == "semaphore" and u.update_value
                            and (u.ant_name or "").startswith("DMA")):
                        k = (u.id, u.ant_name)
                        dma_totals[k] = dma_totals.get(k, 0) + u.update_value
        handles = {h.num: h for h in self.sems.allocated().values()}
        for eng in (nc.gpsimd, nc.sync, nc.tensor, nc.vector, nc.scalar):
            for (sid, _), tot in sorted(dma_totals.items()):
                if sid in handles:
                    eng.wait_ge(handles[sid], tot)
        popped = nc._tile_sem_poison_stack.pop()
        assert popped is self._sem_poison
        nc.clear_and_free_semaphores(list(self.sems.allocated().values()))

BF16 = ml_dtypes.bfloat16
F32 = mybir.dt.float32
BF = mybir.dt.bfloat16

NCORES = 8
KO = 32                       # conv2 output channels per core (256 = 8*32)
# DMA packets (one per partition line) cost ~113ns fixed on a DMA engine
# plus ~size/48GB/s, so few DMAs with long lines beat many short-line
# chunks. Three tensors, sequential on one ring for arrival-order
# gating: c1 first (it gates conv1 and everything after), then all 81
# w2 taps as one 5184B-line transfer, then v (needed last).


N_WARM = 140                  # PE warmup matmuls (p-state ramp + keeps the
                              # PE's first DMA-sem wait shallow; waits on
                              # DMA-posted sems wake ~1.3us late from deep
                              # idle, engine-posted sems wake in ~30ns)
N_FILL_ACT = 6                # ACT keep-warm dummies while conv2 runs
N_FILL_PE = 50                # PE keep-warm dummies while the squash runs

# Shrink the declared kernel semaphore range: the NEFF exit sequence
# clears the whole declared file one EVENT_SEMAPHORE per sem, split
# across engines (~100ns each on the critical engine), so unused
# declared sems cost real epilogue time.
_orig_sem_range = bass.get_kernel_semaphore_range()


def _small_sem_range():
    return range(_orig_sem_range.start, _orig_sem_range.start + 34)


bass.get_kernel_semaphore_range = _small_sem_range


# --------------------------------------------------------------------------
# Host-side input marshalling (pure layout transforms + dtype casts)
# --------------------------------------------------------------------------

def _host_prep(x, conv_w, conv_b, pri_w, pri_b, W):
    x = np.asarray(x, np.float32)
    conv_w = np.asarray(conv_w, np.float32)
    conv_b = np.asarray(conv_b, np.float32)
    pri_w = np.asarray(pri_w, np.float32)
    pri_b = np.asarray(pri_b, np.float32)
    W = np.asarray(W, np.float32)

    # conv1 as 9 dy-taps with contraction (c, dx) + a bias row:
    #   xspread[c*9+dx, y*16+ox] = x[c, y, ox+dx]  (28, 384)
    #   w1r[c*9+dx, dy*128+oc] = conv_w[oc, c, dy, dx]  (28, 1152)
    # packed cx = [xspread | w1r | pb row | ones row]  (28, 1584)
    cx = np.zeros((28, 1584), np.float32)
    for c in range(3):
        for dx in range(9):
            cx[c * 9 + dx, 0:384] = x[0, c, :, dx:dx + 16].reshape(384)
    cx[27, 0:384] = 1.0
    w1r = conv_w.transpose(1, 3, 2, 0).reshape(27, 9 * 128)  # (c,dx),(dy,oc)
    cx[:27, 384:1536] = w1r
    cx[27, 384:512] = conv_b  # dy=0 block of the bias row
    cx[0, 1568:1584] = 1.0

    # conv2 weights (oc2, ic, dy, dx) -> per-core (ic, dydx*32 + o)
    k2 = pri_w.reshape(256, 128, 9, 9)
    pb = pri_b.reshape(256)

    # digitcaps weights with 1/512 folded in:
    #   v[p = 32k + o, c2*160 + ki] = W[0, n, :, :, sp&7]/512 flattened,
    #   sp = 4k + c2, oc2 = 32c + o, n = (oc2>>3)*16 + (oc2&7)*2 + (sp>>3)
    Wd = W[0] * (1.0 / 512.0)  # (512, 10, 16, 8)

    per_core = []
    for c in range(NCORES):
        oc2 = np.arange(c * KO, (c + 1) * KO)
        cxc = cx.copy()
        cxc[0, 1536:1568] = pb[oc2]
        w2c = (k2[oc2]                     # (32, 128, 9, 9)
               .transpose(1, 2, 3, 0)      # (ic, dy, dx, o)
               .reshape(128, 81 * KO))
        v = np.empty((128, 4 * 160), np.float32)
        for k in range(4):
            for c2 in range(4):
                sp = 4 * k + c2
                n = (oc2 >> 3) * 16 + (oc2 & 7) * 2 + (sp >> 3)
                v[32 * k:32 * k + 32, c2 * 160:(c2 + 1) * 160] = \
                    Wd[n, :, :, sp & 7].reshape(KO, 160)
        wbv = np.concatenate([w2c[:, W2SPLIT * KO:], v], axis=1)
        per_core.append({
            "cx": np.ascontiguousarray(cxc).astype(BF16),
            "wa": np.ascontiguousarray(w2c[:, :W2SPLIT * KO]).astype(BF16),
            "wbv": np.ascontiguousarray(wbv).astype(BF16),
        })
    return per_core


W2SPLIT = 41
INPUT_SPECS = {
    "cx": ((28, 1584), BF),
    "wa": ((128, W2SPLIT * KO), BF),
    "wbv": ((128, (81 - W2SPLIT) * KO + 640), BF),
}


# --------------------------------------------------------------------------
# Device IR
# --------------------------------------------------------------------------

def emit_kernel(tc, out_ap, ins):
    nc = tc.nc
    with (
        tc.tile_pool(name="sb", bufs=1) as sb,
        tc.tile_pool(name="ps", bufs=1, space="PSUM") as ps,
    ):
        # ---- input DMAs: sequential on the sync ring in consumption
        # order (concurrent queues would split the shared DMA-engine pool
        # and delay cx, which gates everything).
        cx_sb = sb.tile([28, 1584], BF)
        nc.sync.dma_start(cx_sb[:], ins["cx"][:])
        wa_sb = sb.tile([128, W2SPLIT * KO], BF)
        nc.sync.dma_start(wa_sb[:], ins["wa"][:])
        wbv_sb = sb.tile([128, (81 - W2SPLIT) * KO + 640], BF)
        nc.sync.dma_start(wbv_sb[:], ins["wbv"][:])
        VOFF = (81 - W2SPLIT) * KO

        def w2tap(t):
            if t < W2SPLIT:
                return wa_sb[:, t * KO:(t + 1) * KO]
            return wbv_sb[:, (t - W2SPLIT) * KO:(t - W2SPLIT + 1) * KO]

        # ---- act-table preloads: dummy square/sqrt on const data so the
        # lazy LoadActFuncSet (1.28us each) runs at scalar-engine start
        # instead of stalling the squash chain.
        cf1 = nc.const_aps.aps[(F32, 1.0)]
        dmy = sb.tile([1, 2], F32)
        nc.scalar.square(dmy[:, 0:1], cf1[0:1, :])
        nc.scalar.sqrt(dmy[:, 1:2], cf1[0:1, :])

        # ---- PE warmup: dependency-free 1x1 matmuls on a preamble const
        warm_ps = ps.tile([1, 1], F32)
        cb1 = nc.const_aps.aps[(BF, 1.0)]
        for _ in range(N_WARM):
            nc.tensor.matmul(warm_ps[:], cb1[0:1, :], cb1[0:1, :],
                             start=True, stop=True)

        # ---- conv1 as 9 dy-tap matmuls of contraction 28 (c, dx, bias
        # row) over column windows of xspread -> psum1 (128, 256)
        psum1 = ps.tile([128, 256], F32)
        for dy in range(9):
            nc.tensor.matmul(
                psum1[:],
                cx_sb[:, 384 + dy * 128: 384 + (dy + 1) * 128],
                cx_sb[:, dy * 16: dy * 16 + 256],
                start=(dy == 0), stop=(dy == 8),
            )
        h_sb = sb.tile([128, 256], BF)
        nc.vector.tensor_copy(h_sb[:, 0:128], psum1[:, 0:128])
        nc.scalar.copy(h_sb[:, 128:256], psum1[:, 128:256])
        h4 = h_sb[:].rearrange("p (y x) -> p y x", y=16)

        # ---- conv2 slice: rank-1 pri_b accumulation, then 81 tap matmuls
        # over strided views of h, PSUM-accumulated -> (32, 16) f32
        psum2 = ps.tile([KO, 16], F32)
        nc.tensor.matmul(psum2[:], cx_sb[0:1, 1536:1568],
                         cx_sb[0:1, 1568:1584], start=True, stop=False)
        for dydx in range(81):
            dy, dx = divmod(dydx, 9)
            nc.tensor.matmul(
                psum2[:],
                w2tap(dydx),
                h4[:, dy:dy + 8:2, dx:dx + 8:2],
                start=False, stop=(dydx == 80),
            )

        # keep ACT shallow-idle while conv2 runs (wake on its next wait is
        # ~0.5us late from deep idle)
        for _ in range(N_FILL_ACT):
            nc.scalar.square(dmy[:, 0:1], cf1[0:1, :])

        # ---- squash over groups of 8 along the free dim:
        # u = p / g,  g = (1 + sq) / sqrt(sq),  sq = sum(p^2) per group
        t2 = sb.tile([KO, 16], F32)
        nc.scalar.square(t2[:], psum2[:])
        # stage p to SBUF on ACT (off the DVE critical path) so GpSimd,
        # which cannot read PSUM, can take one of the u-mul blocks
        x2 = sb.tile([KO, 4], F32)
        nc.scalar.copy(x2[:], psum2[:, 12:16])
        sq = sb.tile([KO, 2], F32)
        nc.vector.tensor_reduce(
            sq[:], t2[:].rearrange("p (g e) -> p g e", e=8),
            axis=mybir.AxisListType.X, op=mybir.AluOpType.add,
        )
        r_ = sb.tile([KO, 2], F32)
        nc.scalar.sqrt(r_[:], sq[:])
        d_ = sb.tile([KO, 2], F32)
        nc.vector.tensor_scalar_add(d_[:], sq[:], 1.0)
        rec = sb.tile([KO, 2], F32)
        nc.vector.reciprocal(rec[:], d_[:])
        f_ = sb.tile([KO, 2], F32)
        nc.vector.tensor_mul(f_[:], r_[:], rec[:])
        # u = p * f;
        # u128[p = 32k + o, c2] = u[o, 4k + c2]  (digitcaps-ready layout)
        u128 = sb.tile([128, 4], BF)
        for k in range(3):
            nc.vector.tensor_scalar_mul(
                u128[32 * k:32 * k + 32, :],
                psum2[:, 4 * k:4 * k + 4],
                f_[:, (k >> 1):(k >> 1) + 1],
            )
        nc.gpsimd.tensor_scalar_mul(u128[96:128, :], x2[:], f_[:, 1:2])

        # keep the PE shallow-idle + p-state hot while the squash runs
        for _ in range(N_FILL_PE):
            nc.tensor.matmul(warm_ps[:], cb1[0:1, :], cb1[0:1, :],
                             start=True, stop=True)

        # ---- partial digitcaps: psum_d[0, ki] += u128[:, c2] . v[:, c2]
        psum_d = ps.tile([1, 160], F32)
        for c2 in range(4):
            nc.tensor.matmul(
                psum_d[:],
                u128[:, c2:c2 + 1],
                wbv_sb[:, VOFF + c2 * 160:VOFF + (c2 + 1) * 160],
                start=(c2 == 0), stop=(c2 == 3),
            )

        # ---- store the raw partial sum; the host gather sums the 8
        # partials and applies the final elementwise squash.
        s_sb = sb.tile([1, 160], F32)
        nc.vector.tensor_copy(s_sb[:], psum_d[:])
        nc.sync.dma_start(out_ap[:], s_sb[:], single_packet=True)


# --------------------------------------------------------------------------
# Build + run
# --------------------------------------------------------------------------

_CACHE = {}


def build_nc():
    nc = bacc.Bacc(
        "TRN2", target_bir_lowering=False, debug=False, num_devices=NCORES
    )
    ins = {
        name: nc.dram_tensor(name, list(shape), dt, kind="ExternalInput").ap()
        for name, (shape, dt) in INPUT_SPECS.items()
    }
    out_ap = nc.dram_tensor("out", [1, 160], F32, kind="ExternalOutput").ap()
    with FastTailTileContext(nc) as tc:
        emit_kernel(tc, out_ap, ins)
    nc.compile()
    return nc


def kernel(**inputs):
    per_core = _host_prep(**inputs)
    if "nc" not in _CACHE:
        _CACHE["nc"] = build_nc()
    res = run_bass_kernel_spmd(
        _CACHE["nc"], per_core, core_ids=list(range(NCORES))
    )
    # gather/unshard: all-reduce the partial sums over the sharded
    # input-capsule axis, then the final elementwise squash.
    s = np.zeros(160, np.float64)
    for c in range(NCORES):
        s += np.asarray(res.results[c]["out"], np.float32).reshape(-1)
    s = s.astype(np.float32)
    v = s * np.abs(s) / (1.0 + s * s)
    return v.reshape(1, 1, 10, 16, 1)


# revision 44
# speedup vs baseline: 1.0940x; 1.0940x over previous
"""Trainium2 Bass kernel for nn_CapsNet_69114613730132.

Strategy (8 NeuronCores, SPMD, n-axis tensor parallel per the sharding
hint):
  The CapsNet routing loop is degenerate (self.bij is never updated, so
  cij stays 1/512) and collapses to: conv1 -> conv2 -> squash ->
  4096->160 matvec -> elementwise squash.

  * conv1 (3->128, 9x9, on 24x24) is tiny and replicated: one
    243+1-contraction matmul over a host-built im2col (conv_b folded in
    as an extra contraction row).
  * conv2 / PrimaryCaps is sharded across cores on its 256 output
    channels (= the 512-input-capsule axis of the routing einsum): each
    core holds 32 output channels' weights (1/8 of pri_w, ~660KB bf16
    instead of 5.3MB replicated) and computes its (32, 16) slice of the
    primary capsule grid via 81 PSUM-accumulated tap matmuls over
    strided views of h (pri_b folded in as a rank-1 PE accumulation).
  * squash is applied per-core on its slice (groups of 8 lie along the
    free dim), then each core computes the partial DigitCaps sum over
    its 512-entry slice of the 4096-long contraction for ALL 160
    outputs (4 matmuls of 128-contraction x 160 cols; the constant
    cij=1/512 is folded into the weights).
  * The all-reduce over the input-capsule axis that the sharding hint
    prescribes is realized in the gather/unshard step: the host sums
    the 8 (1,160) partials and applies the final elementwise squash
    s*|s|/(1+s*s) (the routing output for a size-1 capsule dim).
  * All PE compute in bf16 (weights host-cast), f32 PSUM/vector math.

kernel(**inputs) takes the FULL unsharded inputs and returns the full
(1,1,10,16,1) float32 output.
"""
import numpy as np
import ml_dtypes

import concourse.bass as bass
import concourse.bacc as bacc
import concourse.tile as tile
import concourse.mybir as mybir
from concourse.bass_utils import run_bass_kernel_spmd
from concourse.tile import ScopedClock, add_dep_helper

FAST_TAIL = True


class FastTailTileContext(tile.TileContext):
    """TileContext tail with a 1-hop handshake instead of the all-engine
    barriers (each an EVSEM polling butterfly measured at ~7us here).

    The sync.drain waits for every tracked semaphore target, so by the
    time it passes, every sem-touching instruction on every engine has
    retired (each engine's last real work is upstream of the output DMA
    the drain waits on). A single drain->GpSimd semaphore hop then orders
    the sem/DMA-state clears; the next execution's NEFF entry barrier
    orders everything else."""

    def _drain_and_barrier(self, tick_clock, wait_clock):
        if not FAST_TAIL:
            return super()._drain_and_barrier(tick_clock, wait_clock)
        nc = self.nc
        # GpSimd (the clearing engine, otherwise idle here) waits on every
        # tracked semaphore's final value itself, then clears.
        drain_inst = nc.gpsimd.drain()
        wait_clock.add_sem_waits(
            drain_inst.ins, ScopedClock({None: tick_clock.global_clock})
        )
        # DMA completion sems update asynchronously and the final DMA (the
        # output store) has no downstream consumer, so the clock misses it:
        # wait each DMA-lane sem out to its summed final value explicitly.
        dma_totals = {}
        for insts in self.ordered_instructions_by_block.values():
            for i in insts:
                si = i.sync_info
                if si is None or not si.on_update:
                    continue
                for u in si.on_update:
                    if (u.sync_type == "semaphore" and u.update_value
                            and (u.ant_name or "").startswith("DMA")):
                        k = (u.id, u.ant_name)
                        dma_totals[k] = dma_totals.get(k, 0) + u.update_value
        handles = {h.num: h for h in self.sems.allocated().values()}
        for eng in (nc.gpsimd, nc.sync, nc.tensor, nc.vector, nc.scalar):
            for (sid, _), tot in sorted(dma_totals.items()):
                if sid in handles:
                    eng.wait_ge(handles[sid], tot)
        popped = nc._tile_sem_poison_stack.pop()
        assert popped is self._sem_poison
        nc.clear_and_free_semaphores(list(self.sems.allocated().values()))

BF16 = ml_dtypes.bfloat16
F32 = mybir.dt.float32
BF = mybir.dt.bfloat16

NCORES = 8
KO = 32                       # conv2 output channels per core (256 = 8*32)
# DMA packets (one per partition line) cost ~113ns fixed on a DMA engine
# plus ~size/48GB/s, so few DMAs with long lines beat many short-line
# chunks. Three tensors, sequential on one ring for arrival-order
# gating: c1 first (it gates conv1 and everything after), then all 81
# w2 taps as one 5184B-line transfer, then v (needed last).


N_WARM = 125                  # PE warmup matmuls (p-state ramp + keeps the
                              # PE's first DMA-sem wait shallow; waits on
                              # DMA-posted sems wake ~1.3us late from deep
                              # idle, engine-posted sems wake in ~30ns)
N_FILL_ACT = 6                # ACT keep-warm dummies while conv2 runs
N_FILL_PE = 50                # PE keep-warm dummies while the squash runs

# Shrink the declared kernel semaphore range: the NEFF exit sequence
# clears the whole declared file one EVENT_SEMAPHORE per sem, split
# across engines (~100ns each on the critical engine), so unused
# declared sems cost real epilogue time.
_orig_sem_range = bass.get_kernel_semaphore_range()


def _small_sem_range():
    return range(_orig_sem_range.start, _orig_sem_range.start + 34)


bass.get_kernel_semaphore_range = _small_sem_range


# --------------------------------------------------------------------------
# Host-side input marshalling (pure layout transforms + dtype casts)
# --------------------------------------------------------------------------

def _host_prep(x, conv_w, conv_b, pri_w, pri_b, W):
    x = np.asarray(x, np.float32)
    conv_w = np.asarray(conv_w, np.float32)
    conv_b = np.asarray(conv_b, np.float32)
    pri_w = np.asarray(pri_w, np.float32)
    pri_b = np.asarray(pri_b, np.float32)
    W = np.asarray(W, np.float32)

    # conv1 as 9 dy-taps with contraction (c, dx) + a bias row:
    #   xspread[c*9+dx, y*16+ox] = x[c, y, ox+dx]  (28, 384)
    #   w1r[c*9+dx, dy*128+oc] = conv_w[oc, c, dy, dx]  (28, 1152)
    # packed cx = [xspread | w1r | pb row | ones row]  (28, 1584)
    cx = np.zeros((28, 1584), np.float32)
    for c in range(3):
        for dx in range(9):
            cx[c * 9 + dx, 0:384] = x[0, c, :, dx:dx + 16].reshape(384)
    cx[27, 0:384] = 1.0
    w1r = conv_w.transpose(1, 3, 2, 0).reshape(27, 9 * 128)  # (c,dx),(dy,oc)
    cx[:27, 384:1536] = w1r
    cx[27, 384:512] = conv_b  # dy=0 block of the bias row
    cx[0, 1568:1584] = 1.0

    # conv2 weights (oc2, ic, dy, dx) -> per-core (ic, dydx*32 + o)
    k2 = pri_w.reshape(256, 128, 9, 9)
    pb = pri_b.reshape(256)

    # digitcaps weights with 1/512 folded in:
    #   v[p = 32k + o, c2*160 + ki] = W[0, n, :, :, sp&7]/512 flattened,
    #   sp = 4k + c2, oc2 = 32c + o, n = (oc2>>3)*16 + (oc2&7)*2 + (sp>>3)
    Wd = W[0] * (1.0 / 512.0)  # (512, 10, 16, 8)

    per_core = []
    for c in range(NCORES):
        oc2 = np.arange(c * KO, (c + 1) * KO)
        cxc = cx.copy()
        cxc[0, 1536:1568] = pb[oc2]
        w2c = (k2[oc2]                     # (32, 128, 9, 9)
               .transpose(1, 2, 3, 0)      # (ic, dy, dx, o)
               .reshape(128, 81 * KO))
        v = np.empty((128, 4 * 160), np.float32)
        for k in range(4):
            for c2 in range(4):
                sp = 4 * k + c2
                n = (oc2 >> 3) * 16 + (oc2 & 7) * 2 + (sp >> 3)
                v[32 * k:32 * k + 32, c2 * 160:(c2 + 1) * 160] = \
                    Wd[n, :, :, sp & 7].reshape(KO, 160)
        wbv = np.concatenate([w2c[:, W2SPLIT * KO:], v], axis=1)
        per_core.append({
            "cx": np.ascontiguousarray(cxc).astype(BF16),
            "wa": np.ascontiguousarray(w2c[:, :W2SPLIT * KO]).astype(BF16),
            "wbv": np.ascontiguousarray(wbv).astype(BF16),
        })
    return per_core


W2SPLIT = 41
INPUT_SPECS = {
    "cx": ((28, 1584), BF),
    "wa": ((128, W2SPLIT * KO), BF),
    "wbv": ((128, (81 - W2SPLIT) * KO + 640), BF),
}


# --------------------------------------------------------------------------
# Device IR
# --------------------------------------------------------------------------

def emit_kernel(tc, out_ap, ins):
    nc = tc.nc
    with (
        tc.tile_pool(name="sb", bufs=1) as sb,
        tc.tile_pool(name="ps", bufs=1, space="PSUM") as ps,
    ):
        # ---- input DMAs: sequential on the sync ring in consumption
        # order (concurrent queues would split the shared DMA-engine pool
        # and delay cx, which gates everything).
        cx_sb = sb.tile([28, 1584], BF)
        nc.sync.dma_start(cx_sb[:], ins["cx"][:])
        wa_sb = sb.tile([128, W2SPLIT * KO], BF)
        nc.sync.dma_start(wa_sb[:], ins["wa"][:])
        wbv_sb = sb.tile([128, (81 - W2SPLIT) * KO + 640], BF)
        nc.sync.dma_start(wbv_sb[:], ins["wbv"][:])
        VOFF = (81 - W2SPLIT) * KO

        def w2tap(t):
            if t < W2SPLIT:
                return wa_sb[:, t * KO:(t + 1) * KO]
            return wbv_sb[:, (t - W2SPLIT) * KO:(t - W2SPLIT + 1) * KO]

        # ---- act-table preloads: dummy square/sqrt on const data so the
        # lazy LoadActFuncSet (1.28us each) runs at scalar-engine start
        # instead of stalling the squash chain.
        cf1 = nc.const_aps.aps[(F32, 1.0)]
        dmy = sb.tile([1, 2], F32)
        nc.scalar.square(dmy[:, 0:1], cf1[0:1, :])
        nc.scalar.sqrt(dmy[:, 1:2], cf1[0:1, :])

        # ---- PE warmup: dependency-free 1x1 matmuls on a preamble const
        warm_ps = ps.tile([1, 1], F32)
        cb1 = nc.const_aps.aps[(BF, 1.0)]
        for _ in range(N_WARM):
            nc.tensor.matmul(warm_ps[:], cb1[0:1, :], cb1[0:1, :],
                             start=True, stop=True)

        # ---- conv1 as 9 dy-tap matmuls of contraction 28 (c, dx, bias
        # row) over column windows of xspread -> psum1 (128, 256)
        psum1 = ps.tile([128, 256], F32)
        for dy in range(9):
            nc.tensor.matmul(
                psum1[:],
                cx_sb[:, 384 + dy * 128: 384 + (dy + 1) * 128],
                cx_sb[:, dy * 16: dy * 16 + 256],
                start=(dy == 0), stop=(dy == 8),
            )
        h_sb = sb.tile([128, 256], BF)
        nc.vector.tensor_copy(h_sb[:, 0:128], psum1[:, 0:128])
        hcopy = nc.scalar.copy(h_sb[:, 128:256], psum1[:, 128:256])
        h4 = h_sb[:].rearrange("p (y x) -> p y x", y=16)

        # keep ACT shallow-idle while conv2 runs (wake on its next wait is
        # ~0.5us late from deep idle); pinned after the h copy so the tile
        # scheduler cannot hoist them to the program start
        prev = hcopy
        for _ in range(N_FILL_ACT):
            fi = nc.scalar.square(dmy[:, 0:1], cf1[0:1, :])
            add_dep_helper(fi.ins, prev.ins, sync=True,
                           reason="ACT keep-warm filler placement")
            prev = fi

        # ---- conv2 slice: rank-1 pri_b accumulation, then 81 tap matmuls
        # over strided views of h, PSUM-accumulated -> (32, 16) f32
        psum2 = ps.tile([KO, 16], F32)
        nc.tensor.matmul(psum2[:], cx_sb[0:1, 1536:1568],
                         cx_sb[0:1, 1568:1584], start=True, stop=False)
        for dydx in range(81):
            dy, dx = divmod(dydx, 9)
            last_tap = nc.tensor.matmul(
                psum2[:],
                w2tap(dydx),
                h4[:, dy:dy + 8:2, dx:dx + 8:2],
                start=False, stop=(dydx == 80),
            )

        # ---- squash over groups of 8 along the free dim:
        # u = p / g,  g = (1 + sq) / sqrt(sq),  sq = sum(p^2) per group
        t2 = sb.tile([KO, 16], F32)
        nc.scalar.square(t2[:], psum2[:])
        # stage p to SBUF on ACT (off the DVE critical path) so GpSimd,
        # which cannot read PSUM, can take one of the u-mul blocks
        x2 = sb.tile([KO, 4], F32)
        nc.scalar.copy(x2[:], psum2[:, 12:16])
        sq = sb.tile([KO, 2], F32)
        nc.vector.tensor_reduce(
            sq[:], t2[:].rearrange("p (g e) -> p g e", e=8),
            axis=mybir.AxisListType.X, op=mybir.AluOpType.add,
        )
        r_ = sb.tile([KO, 2], F32)
        nc.scalar.sqrt(r_[:], sq[:])
        d_ = sb.tile([KO, 2], F32)
        nc.vector.tensor_scalar_add(d_[:], sq[:], 1.0)
        rec = sb.tile([KO, 2], F32)
        nc.vector.reciprocal(rec[:], d_[:])
        f_ = sb.tile([KO, 2], F32)
        nc.vector.tensor_mul(f_[:], r_[:], rec[:])
        # u = p * f;
        # u128[p = 32k + o, c2] = u[o, 4k + c2]  (digitcaps-ready layout)
        u128 = sb.tile([128, 4], BF)
        for k in range(3):
            nc.vector.tensor_scalar_mul(
                u128[32 * k:32 * k + 32, :],
                psum2[:, 4 * k:4 * k + 4],
                f_[:, (k >> 1):(k >> 1) + 1],
            )
        nc.gpsimd.tensor_scalar_mul(u128[96:128, :], x2[:], f_[:, 1:2])

        # keep the PE shallow-idle + p-state hot while the squash runs;
        # pinned after the last conv2 tap so they fill the gap before the
        # digitcaps matmuls instead of being hoisted to program start
        prev = last_tap
        for _ in range(N_FILL_PE):
            fi = nc.tensor.matmul(warm_ps[:], cb1[0:1, :], cb1[0:1, :],
                                  start=True, stop=True)
            add_dep_helper(fi.ins, prev.ins, sync=True,
                           reason="PE keep-warm filler placement")
            prev = fi

        # ---- partial digitcaps: psum_d[0, ki] += u128[:, c2] . v[:, c2]
        psum_d = ps.tile([1, 160], F32)
        for c2 in range(4):
            nc.tensor.matmul(
                psum_d[:],
                u128[:, c2:c2 + 1],
                wbv_sb[:, VOFF + c2 * 160:VOFF + (c2 + 1) * 160],
                start=(c2 == 0), stop=(c2 == 3),
            )

        # ---- store the raw partial sum; the host gather sums the 8
        # partials and applies the final elementwise squash.
        s_sb = sb.tile([1, 160], F32)
        nc.vector.tensor_copy(s_sb[:], psum_d[:])
        nc.sync.dma_start(out_ap[:], s_sb[:], single_packet=True)


# --------------------------------------------------------------------------
# Build + run
# --------------------------------------------------------------------------

_CACHE = {}


def build_nc():
    nc = bacc.Bacc(
        "TRN2", target_bir_lowering=False, debug=False, num_devices=NCORES
    )
    ins = {
        name: nc.dram_tensor(name, list(shape), dt, kind="ExternalInput").ap()
        for name, (shape, dt) in INPUT_SPECS.items()
    }
    out_ap = nc.dram_tensor("out", [1, 160], F32, kind="ExternalOutput").ap()
    with FastTailTileContext(nc) as tc:
        emit_kernel(tc, out_ap, ins)
    nc.compile()
    return nc


def kernel(**inputs):
    per_core = _host_prep(**inputs)
    if "nc" not in _CACHE:
        _CACHE["nc"] = build_nc()
    res = run_bass_kernel_spmd(
        _CACHE["nc"], per_core, core_ids=list(range(NCORES))
    )
    # gather/unshard: all-reduce the partial sums over the sharded
    # input-capsule axis, then the final elementwise squash.
    s = np.zeros(160, np.float64)
    for c in range(NCORES):
        s += np.asarray(res.results[c]["out"], np.float32).reshape(-1)
    s = s.astype(np.float32)
    v = s * np.abs(s) / (1.0 + s * s)
    return v.reshape(1, 1, 10, 16, 1)


# revision 49
# speedup vs baseline: 1.1043x; 1.0094x over previous
"""Trainium2 Bass kernel for nn_CapsNet_69114613730132.

Strategy (8 NeuronCores, SPMD, n-axis tensor parallel per the sharding
hint):
  The CapsNet routing loop is degenerate (self.bij is never updated, so
  cij stays 1/512) and collapses to: conv1 -> conv2 -> squash ->
  4096->160 matvec -> elementwise squash.

  * conv1 (3->128, 9x9, on 24x24) is tiny and replicated: 9 dy-tap
    matmuls of contraction 28 (channel x dx, plus a bias row folding
    conv_b) over column windows of a host-built x spread -- a 42KB
    input instead of a 122KB im2col, so it lands and starts early.
  * conv2 / PrimaryCaps is sharded across cores on its 256 output
    channels (= the 512-input-capsule axis of the routing einsum): each
    core holds 32 output channels' weights (1/8 of pri_w, ~660KB bf16
    instead of 5.3MB replicated) and computes its (32, 16) slice of the
    primary capsule grid via 81 PSUM-accumulated tap matmuls over
    strided views of h (pri_b folded in as a rank-1 PE accumulation).
  * squash is applied per-core on its slice (groups of 8 lie along the
    free dim), then each core computes the partial DigitCaps sum over
    its 512-entry slice of the 4096-long contraction for ALL 160
    outputs (4 matmuls of 128-contraction x 160 cols; the constant
    cij=1/512 is folded into the weights).
  * The all-reduce over the input-capsule axis that the sharding hint
    prescribes is realized in the gather/unshard step: the host sums
    the 8 (1,160) partials and applies the final elementwise squash
    s*|s|/(1+s*s) (the routing output for a size-1 capsule dim).
  * All PE compute in bf16 (weights host-cast), f32 PSUM/vector math.

kernel(**inputs) takes the FULL unsharded inputs and returns the full
(1,1,10,16,1) float32 output.
"""
import numpy as np
import ml_dtypes

import concourse.bass as bass
import concourse.bacc as bacc
import concourse.tile as tile
import concourse.mybir as mybir
from concourse.bass_utils import run_bass_kernel_spmd
from concourse.tile import ScopedClock

FAST_TAIL = True


class FastTailTileContext(tile.TileContext):
    """TileContext tail with a 1-hop handshake instead of the all-engine
    barriers (each an EVSEM polling butterfly measured at ~7us here).

    The sync.drain waits for every tracked semaphore target, so by the
    time it passes, every sem-touching instruction on every engine has
    retired (each engine's last real work is upstream of the output DMA
    the drain waits on). A single drain->GpSimd semaphore hop then orders
    the sem/DMA-state clears; the next execution's NEFF entry barrier
    orders everything else."""

    def _drain_and_barrier(self, tick_clock, wait_clock):
        if not FAST_TAIL:
            return super()._drain_and_barrier(tick_clock, wait_clock)
        nc = self.nc
        # GpSimd (the clearing engine, otherwise idle here) waits on every
        # tracked semaphore's final value itself, then clears.
        drain_inst = nc.gpsimd.drain()
        wait_clock.add_sem_waits(
            drain_inst.ins, ScopedClock({None: tick_clock.global_clock})
        )
        # DMA completion sems update asynchronously and the final DMA (the
        # output store) has no downstream consumer, so the clock misses it:
        # wait each DMA-lane sem out to its summed final value explicitly.
        dma_totals = {}
        for insts in self.ordered_instructions_by_block.values():
            for i in insts:
                si = i.sync_info
                if si is None or not si.on_update:
                    continue
                for u in si.on_update:
                    if (u.sync_type == "semaphore" and u.update_value
                            and (u.ant_name or "").startswith("DMA")):
                        k = (u.id, u.ant_name)
                        dma_totals[k] = dma_totals.get(k, 0) + u.update_value
        handles = {h.num: h for h in self.sems.allocated().values()}
        for eng in (nc.gpsimd, nc.sync, nc.tensor, nc.vector, nc.scalar):
            for (sid, _), tot in sorted(dma_totals.items()):
                if sid in handles:
                    eng.wait_ge(handles[sid], tot)
        popped = nc._tile_sem_poison_stack.pop()
        assert popped is self._sem_poison
        nc.clear_and_free_semaphores(list(self.sems.allocated().values()))

BF16 = ml_dtypes.bfloat16
F32 = mybir.dt.float32
BF = mybir.dt.bfloat16

NCORES = 8
KO = 32                       # conv2 output channels per core (256 = 8*32)
# DMA packets (one per partition line) cost ~113ns fixed on a DMA engine
# plus ~size/48GB/s, so few DMAs with long lines beat many short-line
# chunks. Three tensors, sequential on one ring for arrival-order
# gating: c1 first (it gates conv1 and everything after), then all 81
# w2 taps as one 5184B-line transfer, then v (needed last).


N_WARM = 60                  # PE warmup matmuls (p-state ramp + keeps the
                              # PE's first DMA-sem wait shallow; waits on
                              # DMA-posted sems wake ~1.3us late from deep
                              # idle, engine-posted sems wake in ~30ns)
N_FILL_ACT = 0                # ACT keep-warm dummies while conv2 runs
N_FILL_PE = 0                # PE keep-warm dummies while the squash runs

# Shrink the declared kernel semaphore range: the NEFF exit sequence
# clears the whole declared file one EVENT_SEMAPHORE per sem, split
# across engines (~100ns each on the critical engine), so unused
# declared sems cost real epilogue time.
_orig_sem_range = bass.get_kernel_semaphore_range()


def _small_sem_range():
    return range(_orig_sem_range.start, _orig_sem_range.start + 34)


bass.get_kernel_semaphore_range = _small_sem_range


# --------------------------------------------------------------------------
# Host-side input marshalling (pure layout transforms + dtype casts)
# --------------------------------------------------------------------------

def _host_prep(x, conv_w, conv_b, pri_w, pri_b, W):
    x = np.asarray(x, np.float32)
    conv_w = np.asarray(conv_w, np.float32)
    conv_b = np.asarray(conv_b, np.float32)
    pri_w = np.asarray(pri_w, np.float32)
    pri_b = np.asarray(pri_b, np.float32)
    W = np.asarray(W, np.float32)

    # conv1 as 9 dy-taps with contraction (c, dx) + a bias row:
    #   xspread[c*9+dx, y*16+ox] = x[c, y, ox+dx]  (28, 384)
    #   w1r[c*9+dx, dy*128+oc] = conv_w[oc, c, dy, dx]  (28, 1152)
    # packed cx = [xspread | w1r | pb row | ones row]  (28, 1584)
    cx = np.zeros((28, 1584), np.float32)
    for c in range(3):
        for dx in range(9):
            cx[c * 9 + dx, 0:384] = x[0, c, :, dx:dx + 16].reshape(384)
    cx[27, 0:384] = 1.0
    w1r = conv_w.transpose(1, 3, 2, 0).reshape(27, 9 * 128)  # (c,dx),(dy,oc)
    cx[:27, 384:1536] = w1r
    cx[27, 384:512] = conv_b  # dy=0 block of the bias row
    cx[0, 1568:1584] = 1.0

    # conv2 weights (oc2, ic, dy, dx) -> per-core (ic, dydx*32 + o)
    k2 = pri_w.reshape(256, 128, 9, 9)
    pb = pri_b.reshape(256)

    # digitcaps weights with 1/512 folded in:
    #   v[p = 32k + o, c2*160 + ki] = W[0, n, :, :, sp&7]/512 flattened,
    #   sp = 4k + c2, oc2 = 32c + o, n = (oc2>>3)*16 + (oc2&7)*2 + (sp>>3)
    Wd = W[0] * (1.0 / 512.0)  # (512, 10, 16, 8)

    per_core = []
    for c in range(NCORES):
        oc2 = np.arange(c * KO, (c + 1) * KO)
        cxc = cx.copy()
        cxc[0, 1536:1568] = pb[oc2]
        w2c = (k2[oc2]                     # (32, 128, 9, 9)
               .transpose(1, 2, 3, 0)      # (ic, dy, dx, o)
               .reshape(128, 81 * KO))
        v = np.empty((128, 4 * 160), np.float32)
        for k in range(4):
            for c2 in range(4):
                sp = 4 * k + c2
                n = (oc2 >> 3) * 16 + (oc2 & 7) * 2 + (sp >> 3)
                v[32 * k:32 * k + 32, c2 * 160:(c2 + 1) * 160] = \
                    Wd[n, :, :, sp & 7].reshape(KO, 160)
        wbv = np.concatenate([w2c[:, W2SPLIT * KO:], v], axis=1)
        per_core.append({
            "cx": np.ascontiguousarray(cxc).astype(BF16),
            "wa": np.ascontiguousarray(w2c[:, :W2SPLIT * KO]).astype(BF16),
            "wbv": np.ascontiguousarray(wbv).astype(BF16),
        })
    return per_core


W2SPLIT = 41
INPUT_SPECS = {
    "cx": ((28, 1584), BF),
    "wa": ((128, W2SPLIT * KO), BF),
    "wbv": ((128, (81 - W2SPLIT) * KO + 640), BF),
}


# --------------------------------------------------------------------------
# Device IR
# --------------------------------------------------------------------------

def emit_kernel(tc, out_ap, ins):
    nc = tc.nc
    with (
        tc.tile_pool(name="sb", bufs=1) as sb,
        tc.tile_pool(name="ps", bufs=1, space="PSUM") as ps,
    ):
        # ---- input DMAs: sequential on the sync ring in consumption
        # order (concurrent queues would split the shared DMA-engine pool
        # and delay cx, which gates everything).
        cx_sb = sb.tile([28, 1584], BF)
        nc.sync.dma_start(cx_sb[:], ins["cx"][:])
        wa_sb = sb.tile([128, W2SPLIT * KO], BF)
        nc.sync.dma_start(wa_sb[:], ins["wa"][:])
        wbv_sb = sb.tile([128, (81 - W2SPLIT) * KO + 640], BF)
        nc.sync.dma_start(wbv_sb[:], ins["wbv"][:])
        VOFF = (81 - W2SPLIT) * KO

        def w2tap(t):
            if t < W2SPLIT:
                return wa_sb[:, t * KO:(t + 1) * KO]
            return wbv_sb[:, (t - W2SPLIT) * KO:(t - W2SPLIT + 1) * KO]

        # ---- act-table preloads: dummy square/sqrt on const data so the
        # lazy LoadActFuncSet (1.28us each) runs at scalar-engine start
        # instead of stalling the squash chain.
        cf1 = nc.const_aps.aps[(F32, 1.0)]
        dmy = sb.tile([1, 2], F32)
        nc.scalar.square(dmy[:, 0:1], cf1[0:1, :])
        nc.scalar.sqrt(dmy[:, 1:2], cf1[0:1, :])

        # ---- PE warmup: dependency-free 1x1 matmuls on a preamble const
        warm_ps = ps.tile([1, 1], F32)
        cb1 = nc.const_aps.aps[(BF, 1.0)]
        for _ in range(N_WARM):
            nc.tensor.matmul(warm_ps[:], cb1[0:1, :], cb1[0:1, :],
                             start=True, stop=True)

        # ---- conv1 as 9 dy-tap matmuls of contraction 28 (c, dx, bias
        # row) over column windows of xspread -> psum1 (128, 256)
        psum1 = ps.tile([128, 256], F32)
        for dy in range(9):
            nc.tensor.matmul(
                psum1[:],
                cx_sb[:, 384 + dy * 128: 384 + (dy + 1) * 128],
                cx_sb[:, dy * 16: dy * 16 + 256],
                start=(dy == 0), stop=(dy == 8),
            )
        h_sb = sb.tile([128, 256], BF)
        nc.vector.tensor_copy(h_sb[:, 0:128], psum1[:, 0:128])
        nc.scalar.copy(h_sb[:, 128:256], psum1[:, 128:256])
        h4 = h_sb[:].rearrange("p (y x) -> p y x", y=16)

        # ---- conv2 slice: rank-1 pri_b accumulation, then 81 tap matmuls
        # over strided views of h, PSUM-accumulated -> (32, 16) f32
        psum2 = ps.tile([KO, 16], F32)
        nc.tensor.matmul(psum2[:], cx_sb[0:1, 1536:1568],
                         cx_sb[0:1, 1568:1584], start=True, stop=False)
        for dydx in range(81):
            dy, dx = divmod(dydx, 9)
            nc.tensor.matmul(
                psum2[:],
                w2tap(dydx),
                h4[:, dy:dy + 8:2, dx:dx + 8:2],
                start=False, stop=(dydx == 80),
            )

        # ---- squash over groups of 8 along the free dim:
        # u = p / g,  g = (1 + sq) / sqrt(sq),  sq = sum(p^2) per group
        t2 = sb.tile([KO, 16], F32)
        nc.scalar.square(t2[:], psum2[:])
        # stage p to SBUF on ACT (off the DVE critical path) so GpSimd,
        # which cannot read PSUM, can take one of the u-mul blocks
        x2 = sb.tile([KO, 4], F32)
        nc.scalar.copy(x2[:], psum2[:, 12:16])
        sq = sb.tile([KO, 2], F32)
        nc.vector.tensor_reduce(
            sq[:], t2[:].rearrange("p (g e) -> p g e", e=8),
            axis=mybir.AxisListType.X, op=mybir.AluOpType.add,
        )
        r_ = sb.tile([KO, 2], F32)
        nc.scalar.sqrt(r_[:], sq[:])
        d_ = sb.tile([KO, 2], F32)
        nc.vector.tensor_scalar_add(d_[:], sq[:], 1.0)
        rec = sb.tile([KO, 2], F32)
        nc.vector.reciprocal(rec[:], d_[:])
        # u = (p * sqrt(sq)) * (1/(1+sq)) in one dual-scalar op per block;
        # u128[p = 32k + o, c2] = u[o, 4k + c2]  (digitcaps-ready layout)
        u128 = sb.tile([128, 4], BF)
        for k in range(3):
            nc.vector.tensor_scalar(
                u128[32 * k:32 * k + 32, :],
                psum2[:, 4 * k:4 * k + 4],
                r_[:, (k >> 1):(k >> 1) + 1],
                rec[:, (k >> 1):(k >> 1) + 1],
                op0=mybir.AluOpType.mult, op1=mybir.AluOpType.mult,
            )
        nc.gpsimd.tensor_scalar(
            u128[96:128, :], x2[:], r_[:, 1:2], rec[:, 1:2],
            op0=mybir.AluOpType.mult, op1=mybir.AluOpType.mult,
        )

        # ---- partial digitcaps: psum_d[0, ki] += u128[:, c2] . v[:, c2]
        psum_d = ps.tile([1, 160], F32)
        for c2 in range(4):
            nc.tensor.matmul(
                psum_d[:],
                u128[:, c2:c2 + 1],
                wbv_sb[:, VOFF + c2 * 160:VOFF + (c2 + 1) * 160],
                start=(c2 == 0), stop=(c2 == 3),
            )

        # ---- store the raw partial sum; the host gather sums the 8
        # partials and applies the final elementwise squash.
        s_sb = sb.tile([1, 160], F32)
        nc.vector.tensor_copy(s_sb[:], psum_d[:])
        nc.sync.dma_start(out_ap[:], s_sb[:], single_packet=True)


# --------------------------------------------------------------------------
# Build + run
# --------------------------------------------------------------------------

_CACHE = {}


def build_nc():
    nc = bacc.Bacc(
        "TRN2", target_bir_lowering=False, debug=False, num_devices=NCORES
    )
    ins = {
        name: nc.dram_tensor(name, list(shape), dt, kind="ExternalInput").ap()
        for name, (shape, dt) in INPUT_SPECS.items()
    }
    out_ap = nc.dram_tensor("out", [1, 160], F32, kind="ExternalOutput").ap()
    with FastTailTileContext(nc) as tc:
        emit_kernel(tc, out_ap, ins)
    nc.compile()
    return nc


def kernel(**inputs):
    per_core = _host_prep(**inputs)
    if "nc" not in _CACHE:
        _CACHE["nc"] = build_nc()
    res = run_bass_kernel_spmd(
        _CACHE["nc"], per_core, core_ids=list(range(NCORES))
    )
    # gather/unshard: all-reduce the partial sums over the sharded
    # input-capsule axis, then the final elementwise squash.
    s = np.zeros(160, np.float64)
    for c in range(NCORES):
        s += np.asarray(res.results[c]["out"], np.float32).reshape(-1)
    s = s.astype(np.float32)
    v = s * np.abs(s) / (1.0 + s * s)
    return v.reshape(1, 1, 10, 16, 1)


# revision 52
# speedup vs baseline: 1.1179x; 1.0124x over previous
"""Trainium2 Bass kernel for nn_CapsNet_69114613730132.

Strategy (8 NeuronCores, SPMD, n-axis tensor parallel per the sharding
hint):
  The CapsNet routing loop is degenerate (self.bij is never updated, so
  cij stays 1/512) and collapses to: conv1 -> conv2 -> squash ->
  4096->160 matvec -> elementwise squash.

  * conv1 (3->128, 9x9, on 24x24) is tiny and replicated: 9 dy-tap
    matmuls of contraction 28 (channel x dx, plus a bias row folding
    conv_b) over column windows of a host-built x spread -- a 42KB
    input instead of a 122KB im2col, so it lands and starts early.
  * conv2 / PrimaryCaps is sharded across cores on its 256 output
    channels (= the 512-input-capsule axis of the routing einsum): each
    core holds 32 output channels' weights (1/8 of pri_w, ~660KB bf16
    instead of 5.3MB replicated) and computes its (32, 16) slice of the
    primary capsule grid via 81 PSUM-accumulated tap matmuls over
    strided views of h (pri_b folded in as a rank-1 PE accumulation).
  * squash is applied per-core on its slice (groups of 8 lie along the
    free dim), then each core computes the partial DigitCaps sum over
    its 512-entry slice of the 4096-long contraction for ALL 160
    outputs (4 matmuls of 128-contraction x 160 cols; the constant
    cij=1/512 is folded into the weights).
  * The all-reduce over the input-capsule axis that the sharding hint
    prescribes is realized in the gather/unshard step: the host sums
    the 8 (1,160) partials and applies the final elementwise squash
    s*|s|/(1+s*s) (the routing output for a size-1 capsule dim).
  * All PE compute in bf16 (weights host-cast), f32 PSUM/vector math.

kernel(**inputs) takes the FULL unsharded inputs and returns the full
(1,1,10,16,1) float32 output.
"""
import numpy as np
import ml_dtypes

import concourse.bass as bass
import concourse.bacc as bacc
import concourse.tile as tile
import concourse.mybir as mybir
from concourse.bass_utils import run_bass_kernel_spmd
from concourse.tile import ScopedClock

FAST_TAIL = True


class FastTailTileContext(tile.TileContext):
    """TileContext tail with a 1-hop handshake instead of the all-engine
    barriers (each an EVSEM polling butterfly measured at ~7us here).

    The sync.drain waits for every tracked semaphore target, so by the
    time it passes, every sem-touching instruction on every engine has
    retired (each engine's last real work is upstream of the output DMA
    the drain waits on). A single drain->GpSimd semaphore hop then orders
    the sem/DMA-state clears; the next execution's NEFF entry barrier
    orders everything else."""

    def _drain_and_barrier(self, tick_clock, wait_clock):
        if not FAST_TAIL:
            return super()._drain_and_barrier(tick_clock, wait_clock)
        nc = self.nc
        # GpSimd (the clearing engine, otherwise idle here) waits on every
        # tracked semaphore's final value itself, then clears.
        drain_inst = nc.gpsimd.drain()
        wait_clock.add_sem_waits(
            drain_inst.ins, ScopedClock({None: tick_clock.global_clock})
        )
        # DMA completion sems update asynchronously and the final DMA (the
        # output store) has no downstream consumer, so the clock misses it:
        # wait each DMA-lane sem out to its summed final value explicitly.
        dma_totals = {}
        for insts in self.ordered_instructions_by_block.values():
            for i in insts:
                si = i.sync_info
                if si is None or not si.on_update:
                    continue
                for u in si.on_update:
                    if (u.sync_type == "semaphore" and u.update_value
                            and (u.ant_name or "").startswith("DMA")):
                        k = (u.id, u.ant_name)
                        dma_totals[k] = dma_totals.get(k, 0) + u.update_value
        handles = {h.num: h for h in self.sems.allocated().values()}
        for eng in (nc.gpsimd, nc.sync, nc.tensor, nc.vector, nc.scalar):
            for (sid, _), tot in sorted(dma_totals.items()):
                if sid in handles:
                    eng.wait_ge(handles[sid], tot)
        popped = nc._tile_sem_poison_stack.pop()
        assert popped is self._sem_poison
        nc.clear_and_free_semaphores(list(self.sems.allocated().values()))

BF16 = ml_dtypes.bfloat16
F32 = mybir.dt.float32
BF = mybir.dt.bfloat16

NCORES = 8
KO = 32                       # conv2 output channels per core (256 = 8*32)
# DMA packets (one per partition line) cost ~113ns fixed on a DMA engine
# plus ~size/48GB/s, so few DMAs with long lines beat many short-line
# chunks. Three tensors, sequential on one ring for arrival-order
# gating: c1 first (it gates conv1 and everything after), then all 81
# w2 taps as one 5184B-line transfer, then v (needed last).


N_WARM = 60                  # PE warmup matmuls (p-state ramp + keeps the
                              # PE's first DMA-sem wait shallow; waits on
                              # DMA-posted sems wake ~1.3us late from deep
                              # idle, engine-posted sems wake in ~30ns)
N_FILL_ACT = 0                # ACT keep-warm dummies while conv2 runs
N_FILL_PE = 0                # PE keep-warm dummies while the squash runs

# Shrink the declared kernel semaphore range: the NEFF exit sequence
# clears the whole declared file one EVENT_SEMAPHORE per sem, split
# across engines (~100ns each on the critical engine), so unused
# declared sems cost real epilogue time.
_orig_sem_range = bass.get_kernel_semaphore_range()


def _small_sem_range():
    return range(_orig_sem_range.start, _orig_sem_range.start + 34)


bass.get_kernel_semaphore_range = _small_sem_range


# --------------------------------------------------------------------------
# Host-side input marshalling (pure layout transforms + dtype casts)
# --------------------------------------------------------------------------

def _host_prep(x, conv_w, conv_b, pri_w, pri_b, W):
    x = np.asarray(x, np.float32)
    conv_w = np.asarray(conv_w, np.float32)
    conv_b = np.asarray(conv_b, np.float32)
    pri_w = np.asarray(pri_w, np.float32)
    pri_b = np.asarray(pri_b, np.float32)
    W = np.asarray(W, np.float32)

    # conv1 as 9 dy-taps with contraction (c, dx) + a bias row:
    #   xspread[c*9+dx, y*16+ox] = x[c, y, ox+dx]  (28, 384)
    #   w1r[c*9+dx, dy*128+oc] = conv_w[oc, c, dy, dx]  (28, 1152)
    # packed cx = [xspread | w1r | pb row | ones row]  (28, 1584)
    cx = np.zeros((28, 1584), np.float32)
    for c in range(3):
        for dx in range(9):
            cx[c * 9 + dx, 0:384] = x[0, c, :, dx:dx + 16].reshape(384)
    cx[27, 0:384] = 1.0
    w1r = conv_w.transpose(1, 3, 2, 0).reshape(27, 9 * 128)  # (c,dx),(dy,oc)
    cx[:27, 384:1536] = w1r
    cx[27, 384:512] = conv_b  # dy=0 block of the bias row
    cx[0, 1568:1584] = 1.0

    # conv2 weights (oc2, ic, dy, dx) -> per-core (ic, dydx*32 + o)
    k2 = pri_w.reshape(256, 128, 9, 9)
    pb = pri_b.reshape(256)

    # digitcaps weights with 1/512 folded in:
    #   v[p = 32k + o, c2*160 + ki] = W[0, n, :, :, sp&7]/512 flattened,
    #   sp = 4k + c2, oc2 = 32c + o, n = (oc2>>3)*16 + (oc2&7)*2 + (sp>>3)
    Wd = W[0] * (1.0 / 512.0)  # (512, 10, 16, 8)

    per_core = []
    for c in range(NCORES):
        oc2 = np.arange(c * KO, (c + 1) * KO)
        cxc = cx.copy()
        cxc[0, 1536:1568] = pb[oc2]
        w2c = (k2[oc2]                     # (32, 128, 9, 9)
               .transpose(1, 2, 3, 0)      # (ic, dy, dx, o)
               .reshape(128, 81 * KO))
        v = np.empty((128, 4 * 160), np.float32)
        for k in range(4):
            for c2 in range(4):
                sp = 4 * k + c2
                n = (oc2 >> 3) * 16 + (oc2 & 7) * 2 + (sp >> 3)
                v[32 * k:32 * k + 32, c2 * 160:(c2 + 1) * 160] = \
                    Wd[n, :, :, sp & 7].reshape(KO, 160)
        wbv = np.concatenate([w2c[:, W2SPLIT * KO:], v], axis=1)
        per_core.append({
            "cx": np.ascontiguousarray(cxc).astype(BF16),
            "wa": np.ascontiguousarray(w2c[:, :W2SPLIT * KO]).astype(BF16),
            "wbv": np.ascontiguousarray(wbv).astype(BF16),
        })
    return per_core


W2SPLIT = 55
INPUT_SPECS = {
    "cx": ((28, 1584), BF),
    "wa": ((128, W2SPLIT * KO), BF),
    "wbv": ((128, (81 - W2SPLIT) * KO + 640), BF),
}


# --------------------------------------------------------------------------
# Device IR
# --------------------------------------------------------------------------

def emit_kernel(tc, out_ap, ins):
    nc = tc.nc
    with (
        tc.tile_pool(name="sb", bufs=1) as sb,
        tc.tile_pool(name="ps", bufs=1, space="PSUM") as ps,
    ):
        # ---- input DMAs: sequential on the sync ring in consumption
        # order (concurrent queues would split the shared DMA-engine pool
        # and delay cx, which gates everything).
        cx_sb = sb.tile([28, 1584], BF)
        nc.sync.dma_start(cx_sb[:], ins["cx"][:])
        wa_sb = sb.tile([128, W2SPLIT * KO], BF)
        nc.sync.dma_start(wa_sb[:], ins["wa"][:])
        wbv_sb = sb.tile([128, (81 - W2SPLIT) * KO + 640], BF)
        nc.sync.dma_start(wbv_sb[:], ins["wbv"][:])
        VOFF = (81 - W2SPLIT) * KO

        def w2tap(t):
            if t < W2SPLIT:
                return wa_sb[:, t * KO:(t + 1) * KO]
            return wbv_sb[:, (t - W2SPLIT) * KO:(t - W2SPLIT + 1) * KO]

        # ---- act-table preloads: dummy square/sqrt on const data so the
        # lazy LoadActFuncSet (1.28us each) runs at scalar-engine start
        # instead of stalling the squash chain.
        cf1 = nc.const_aps.aps[(F32, 1.0)]
        dmy = sb.tile([1, 2], F32)
        nc.scalar.square(dmy[:, 0:1], cf1[0:1, :])
        nc.scalar.sqrt(dmy[:, 1:2], cf1[0:1, :])

        # ---- PE warmup: dependency-free 1x1 matmuls on a preamble const
        warm_ps = ps.tile([1, 1], F32)
        cb1 = nc.const_aps.aps[(BF, 1.0)]
        for _ in range(N_WARM):
            nc.tensor.matmul(warm_ps[:], cb1[0:1, :], cb1[0:1, :],
                             start=True, stop=True)

        # ---- conv1 as 9 dy-tap matmuls of contraction 28 (c, dx, bias
        # row) over column windows of xspread -> psum1 (128, 256)
        psum1 = ps.tile([128, 256], F32)
        for dy in range(9):
            nc.tensor.matmul(
                psum1[:],
                cx_sb[:, 384 + dy * 128: 384 + (dy + 1) * 128],
                cx_sb[:, dy * 16: dy * 16 + 256],
                start=(dy == 0), stop=(dy == 8),
            )
        h_sb = sb.tile([128, 256], BF)
        nc.vector.tensor_copy(h_sb[:, 0:128], psum1[:, 0:128])
        nc.scalar.copy(h_sb[:, 128:256], psum1[:, 128:256])
        h4 = h_sb[:].rearrange("p (y x) -> p y x", y=16)

        # ---- conv2 slice: rank-1 pri_b accumulation, then 81 tap matmuls
        # over strided views of h, PSUM-accumulated -> (32, 16) f32
        psum2 = ps.tile([KO, 16], F32)
        nc.tensor.matmul(psum2[:], cx_sb[0:1, 1536:1568],
                         cx_sb[0:1, 1568:1584], start=True, stop=False)
        for dydx in range(81):
            dy, dx = divmod(dydx, 9)
            nc.tensor.matmul(
                psum2[:],
                w2tap(dydx),
                h4[:, dy:dy + 8:2, dx:dx + 8:2],
                start=False, stop=(dydx == 80),
            )

        # ---- squash over groups of 8 along the free dim:
        # u = p / g,  g = (1 + sq) / sqrt(sq),  sq = sum(p^2) per group
        t2 = sb.tile([KO, 16], F32)
        nc.scalar.square(t2[:], psum2[:])
        # stage p to SBUF on ACT (off the DVE critical path) so GpSimd,
        # which cannot read PSUM, can take one of the u-mul blocks
        x2 = sb.tile([KO, 4], F32)
        nc.scalar.copy(x2[:], psum2[:, 12:16])
        sq = sb.tile([KO, 2], F32)
        nc.vector.tensor_reduce(
            sq[:], t2[:].rearrange("p (g e) -> p g e", e=8),
            axis=mybir.AxisListType.X, op=mybir.AluOpType.add,
        )
        r_ = sb.tile([KO, 2], F32)
        nc.scalar.sqrt(r_[:], sq[:])
        d_ = sb.tile([KO, 2], F32)
        nc.vector.tensor_scalar_add(d_[:], sq[:], 1.0)
        rec = sb.tile([KO, 2], F32)
        nc.vector.reciprocal(rec[:], d_[:])
        # u = (p * sqrt(sq)) * (1/(1+sq)) in one dual-scalar op per block;
        # u128[p = 32k + o, c2] = u[o, 4k + c2]  (digitcaps-ready layout)
        u128 = sb.tile([128, 4], BF)
        for k in range(3):
            nc.vector.tensor_scalar(
                u128[32 * k:32 * k + 32, :],
                psum2[:, 4 * k:4 * k + 4],
                r_[:, (k >> 1):(k >> 1) + 1],
                rec[:, (k >> 1):(k >> 1) + 1],
                op0=mybir.AluOpType.mult, op1=mybir.AluOpType.mult,
            )
        nc.gpsimd.tensor_scalar(
            u128[96:128, :], x2[:], r_[:, 1:2], rec[:, 1:2],
            op0=mybir.AluOpType.mult, op1=mybir.AluOpType.mult,
        )

        # ---- partial digitcaps: psum_d[0, ki] += u128[:, c2] . v[:, c2]
        psum_d = ps.tile([1, 160], F32)
        for c2 in range(4):
            nc.tensor.matmul(
                psum_d[:],
                u128[:, c2:c2 + 1],
                wbv_sb[:, VOFF + c2 * 160:VOFF + (c2 + 1) * 160],
                start=(c2 == 0), stop=(c2 == 3),
            )

        # ---- store the raw partial sum; the host gather sums the 8
        # partials and applies the final elementwise squash.
        s_sb = sb.tile([1, 160], F32)
        nc.vector.tensor_copy(s_sb[:], psum_d[:])
        nc.sync.dma_start(out_ap[:], s_sb[:], single_packet=True)


# --------------------------------------------------------------------------
# Build + run
# --------------------------------------------------------------------------

_CACHE = {}


def build_nc():
    nc = bacc.Bacc(
        "TRN2", target_bir_lowering=False, debug=False, num_devices=NCORES
    )
    ins = {
        name: nc.dram_tensor(name, list(shape), dt, kind="ExternalInput").ap()
        for name, (shape, dt) in INPUT_SPECS.items()
    }
    out_ap = nc.dram_tensor("out", [1, 160], F32, kind="ExternalOutput").ap()
    with FastTailTileContext(nc) as tc:
        emit_kernel(tc, out_ap, ins)
    nc.compile()
    return nc


def kernel(**inputs):
    per_core = _host_prep(**inputs)
    if "nc" not in _CACHE:
        _CACHE["nc"] = build_nc()
    res = run_bass_kernel_spmd(
        _CACHE["nc"], per_core, core_ids=list(range(NCORES))
    )
    # gather/unshard: all-reduce the partial sums over the sharded
    # input-capsule axis, then the final elementwise squash.
    s = np.zeros(160, np.float64)
    for c in range(NCORES):
        s += np.asarray(res.results[c]["out"], np.float32).reshape(-1)
    s = s.astype(np.float32)
    v = s * np.abs(s) / (1.0 + s * s)
    return v.reshape(1, 1, 10, 16, 1)
